# revision 3
# baseline (speedup 1.0000x reference)
"""Trainium2 Bass kernel for CoreAttentionExpand (sparse local+global attention).

Sharding: tensor-parallel over heads. 16 heads / 8 cores = 2 heads per core.
Each core computes RoPE + local-block attention + pooled-global attention for
its 2 heads end-to-end (no collectives); host reassembles the full output.

Layout notes (vs. the original f32 kernel, ~181us -> ~141us in TimelineSim):
- All device data is bf16 (host pre-casts); scores/PV accumulate in f32 PSUM.
  Error budget: rel err ~5e-3 vs the 2e-2 tolerance.
- RoPE = 3 bf16 tensor_tensor ops on DVE per chunk (2x DVE mode); the
  rotate-half operand is a host-prepared HBM copy (no on-chip shuffle).
- Softmax normalization: reciprocal (PSUM->SBUF) + multiply; walrus allows at
  most one PSUM operand per TensorTensor, and gpsimd may not touch PSUM.
- V is loaded partition-major (full-speed DMA descriptors); K/V global pooling
  is streamed per block (causal); per-block output DMAs shrink the tail.
- Output is bf16 [head, D, T]; host converts to f32 and reassembles.
"""

import sys
import math

_REPO = "/opt/trn_rl_repo"
if _REPO not in sys.path:
    sys.path.insert(0, _REPO)

import numpy as np
import ml_dtypes

# ---------------------------------------------------------------- constants
H = 16          # heads
D = 128         # head dim
T = 4096        # tokens
L = 1024        # local block size
S = 128         # global pool stride
E = 128         # local history size
PNUM = T // L   # 4 local blocks
KLEN = T // S + 1  # 33 global keys (incl. zero token)
NT = T // 128   # 32 token-tiles per head
NCORES = 8
HPC = H // NCORES  # 2 heads per core
NEGBIG = -10000.0
SCALE = 1.0 / math.sqrt(D)
CHUNK = 512     # global q-chunk width

_CACHE = {}
_BF = ml_dtypes.bfloat16


def _apply_framework_patches(bassmod, mybir, tilemod):
    """This walrus build rejects >1 sem wait per instruction; split excess
    waits onto preceding same-engine NoOps (pure scheduling transform)."""
    if getattr(tilemod.TileContext, "_wait_split_patched", False):
        return
    TileContext = tilemod.TileContext
    ScopedClock = tilemod.ScopedClock

    orig_add = TileContext._add_instruction
    ctr = [0]

    def split_add(self, inst):
        si = inst.sync_info
        if si is not None and si.on_wait and len(si.on_wait) > 1:
            ow = list(si.on_wait)
            for w in ow[:-1]:
                ctr[0] += 1
                nop = mybir.InstNoOp(name=f"I-wsplit{ctr[0]}", engine=inst.engine)
                nop.sync_info = mybir.SyncInfo(on_wait=[w], on_update=[])
                orig_add(self, nop)
            si.on_wait = [ow[-1]]
        orig_add(self, inst)

    def drain_and_barrier(self, tick_clock, wait_clock):
        nc = self.nc
        probe = nc.sync.nop(nofuse=True, hint="waitprobe")
        wait_clock.add_sem_waits(
            probe.ins, ScopedClock({None: tick_clock.global_clock})
        )
        si = probe.ins.sync_info
        ow = list(si.on_wait) if si and si.on_wait else []
        if len(ow) > 1:
            si.on_wait = ow[:1]
            for w in ow[1:]:
                n2 = nc.sync.nop(nofuse=True, hint="waitsplit")
                n2.ins.sync_info = mybir.SyncInfo(on_wait=[w], on_update=[])
        nc.sync.drain()
        nc.all_engine_barrier()
        popped = nc._tile_sem_poison_stack.pop()
        assert popped is self._sem_poison
        nc.clear_and_free_semaphores(list(self.sems.allocated().values()))
        nc.all_engine_barrier()

    TileContext._add_instruction = split_add
    TileContext._drain_and_barrier = drain_and_barrier
    TileContext._wait_split_patched = True


# ---------------------------------------------------------------- constants (host)
def _host_constants():
    t = np.arange(T, dtype=np.float32)
    inv = (1.0 / (10000.0 ** (np.arange(0, D, 2, dtype=np.float32) / D))).astype(
        np.float32
    )  # [64]
    emb = t[:, None] * inv[None, :]          # [T, 64]
    cos64 = np.cos(emb).astype(np.float32)
    sin64 = np.sin(emb).astype(np.float32)
    # [D, T] head-dim-major tables (sign of rotate-half folded into sinRT)
    cosT = np.ascontiguousarray(np.concatenate([cos64, cos64], axis=1).T).astype(_BF)
    sinRT = np.ascontiguousarray(np.concatenate([-sin64, sin64], axis=1).T).astype(_BF)

    idx = np.arange(128)
    # causal ramp: (mB^T mC)[k, q] = NEGBIG * max(k - q, 0)
    mB = (idx[:, None] <= idx[None, :]).astype(_BF)              # [m, k]: m <= k
    mC = (NEGBIG * (idx[:, None] > idx[None, :])).astype(_BF)    # [m, q]: m > q
    ones_bf = np.ones((128, 128), dtype=_BF)
    # global stairstep: for chunk c, row j, masked cols qq < 128*(j - 4c)
    gB = np.zeros((3, 8 * KLEN), dtype=np.float32)
    for c in range(8):
        for mm in range(3):
            for j in range(KLEN):
                gB[mm, KLEN * c + j] = 1.0 if j >= 4 * c + mm + 1 else 0.0
    gB = gB.astype(_BF)
    qq = np.arange(CHUNK)
    gC = np.stack(
        [NEGBIG * ((qq >= 128 * mm) & (qq < 128 * (mm + 1))) for mm in range(3)]
    ).astype(_BF)                                                # [3, 512]
    ident = np.eye(128, dtype=np.float32)
    poolcol = np.full((128, 1), 1.0 / S, dtype=_BF)              # pooling matmul rhs
    return {
        "cosT": cosT,
        "sinRT": sinRT,
        "mB": mB,
        "mC": mC,
        "ones_bf": ones_bf,
        "gB": gB,
        "gC": gC,
        "ident": ident,
        "poolcol": poolcol,
    }


# ---------------------------------------------------------------- device program
def _build_program():
    import concourse.bass as bass
    import concourse.mybir as mybir
    import concourse.tile as tile

    _apply_framework_patches(bass, mybir, tile)

    f32 = mybir.dt.float32
    bf16 = mybir.dt.bfloat16
    EXP = mybir.ActivationFunctionType.Exp
    MUL = mybir.AluOpType.mult
    ADD = mybir.AluOpType.add
    DIV = mybir.AluOpType.divide

    nc = bass.Bass()
    qT_d = nc.dram_tensor("qT", [HPC, D, T], bf16, kind="ExternalInput")
    qR_d = nc.dram_tensor("qR", [HPC, D, T], bf16, kind="ExternalInput")
    kT_d = nc.dram_tensor("kT", [HPC, D, T], bf16, kind="ExternalInput")
    kR_d = nc.dram_tensor("kR", [HPC, D, T], bf16, kind="ExternalInput")
    v_d = nc.dram_tensor("vP", [HPC, 128, NT, D], bf16, kind="ExternalInput")
    zk_d = nc.dram_tensor("zk", [HPC, D, 1], f32, kind="ExternalInput")
    zv_d = nc.dram_tensor("zv", [HPC, D, 1], f32, kind="ExternalInput")
    cosT_d = nc.dram_tensor("cosT", [D, T], bf16, kind="ExternalInput")
    sinRT_d = nc.dram_tensor("sinRT", [D, T], bf16, kind="ExternalInput")
    mB_d = nc.dram_tensor("mB", [128, 128], bf16, kind="ExternalInput")
    mC_d = nc.dram_tensor("mC", [128, 128], bf16, kind="ExternalInput")
    ones_d = nc.dram_tensor("ones_bf", [128, 128], bf16, kind="ExternalInput")
    gB_d = nc.dram_tensor("gB", [3, 8 * KLEN], bf16, kind="ExternalInput")
    gC_d = nc.dram_tensor("gC", [3, CHUNK], bf16, kind="ExternalInput")
    poolcol_d = nc.dram_tensor("poolcol", [128, 1], bf16, kind="ExternalInput")
    ident_d = nc.dram_tensor("ident", [128, 128], f32, kind="ExternalInput")
    out_d = nc.dram_tensor("outT", [HPC, D, T], bf16, kind="ExternalOutput")

    with tile.TileContext(nc) as tc:
        with (
            tc.tile_pool(name="persist", bufs=1) as persist,
            tc.tile_pool(name="ropebuf", bufs=2) as ropebuf,
            tc.tile_pool(name="ropetmp", bufs=2) as ropetmp,
            tc.tile_pool(name="expp", bufs=12) as expp,
            tc.tile_pool(name="small", bufs=2) as small,
            tc.tile_pool(name="combine", bufs=2) as combine,
            tc.tile_pool(name="scores", bufs=2, space="PSUM") as scores_p,
            tc.tile_pool(name="acc", bufs=1, space="PSUM") as acc_p,
        ):
            # ---- tiles
            mB = persist.tile([128, 128], bf16, tag="mB")
            mC = persist.tile([128, 128], bf16, tag="mC")
            ones_bf = persist.tile([128, 128], bf16, tag="ones")
            gB = persist.tile([3, 8 * KLEN], bf16, tag="gB")
            gC = persist.tile([3, CHUNK], bf16, tag="gC")
            poolcol = persist.tile([128, 1], bf16, tag="poolcol")
            ident = persist.tile([128, 128], f32, tag="ident")
            cosT = persist.tile([D, T], bf16, tag="cosT")
            sinRT = persist.tile([D, T], bf16, tag="sinRT")
            zk0 = persist.tile([D, 1], f32, tag="zk0")
            zk1 = persist.tile([D, 1], f32, tag="zk1")

            QT, KT, VBF, KGT, VGA, VG, OUTSB = {}, {}, {}, {}, {}, {}, {}
            raws = {}
            for h in range(HPC):
                QT[h] = persist.tile([D, T], bf16, tag=f"QT{h}", name=f"QT{h}")
                KT[h] = persist.tile([D, T], bf16, tag=f"KT{h}", name=f"KT{h}")
                VBF[h] = persist.tile([128, NT, D], bf16, tag=f"vbf{h}",
                                      name=f"VBF{h}")
                KGT[h] = persist.tile([D, KLEN], bf16, tag=f"kgT{h}",
                                      name=f"KGT{h}")
                VGA[h] = persist.tile([D, KLEN], f32, tag=f"vgA{h}",
                                      name=f"VGA{h}")
                VG[h] = persist.tile([KLEN, 128], bf16, tag=f"Vg{h}",
                                     name=f"VG{h}")
                OUTSB[h] = persist.tile([D, T], bf16, tag=f"out{h}",
                                        name=f"OUTSB{h}")
                raws[h] = (
                    ropebuf.tile([D, T], bf16, tag="qraw", name=f"qraw{h}"),
                    ropebuf.tile([D, T], bf16, tag="qrot", name=f"qrot{h}"),
                    ropebuf.tile([D, T], bf16, tag="kraw", name=f"krawt{h}"),
                    ropebuf.tile([D, T], bf16, tag="krot", name=f"krot{h}"),
                )

            # ---- DMA issue order: h0 critical path first, one DMA per tensor
            nc.sync.dma_start(out=cosT, in_=cosT_d.ap())
            nc.sync.dma_start(out=sinRT, in_=sinRT_d.ap())
            for tl_, src in zip(raws[0], (qT_d, qR_d, kT_d, kR_d)):
                nc.sync.dma_start(out=tl_, in_=src.ap()[0])
            nc.sync.dma_start(out=mB, in_=mB_d.ap())
            nc.sync.dma_start(out=mC, in_=mC_d.ap())
            nc.sync.dma_start(out=ones_bf, in_=ones_d.ap())
            nc.sync.dma_start(out=VBF[0], in_=v_d.ap()[0])
            nc.sync.dma_start(out=gB, in_=gB_d.ap())
            nc.sync.dma_start(out=gC, in_=gC_d.ap())
            nc.sync.dma_start(out=poolcol, in_=poolcol_d.ap())
            nc.sync.dma_start(out=ident, in_=ident_d.ap())
            nc.sync.dma_start(out=zk0, in_=zk_d.ap()[0])
            nc.sync.dma_start(out=VGA[0][:, 0:1], in_=zv_d.ap()[0])
            for tl_, src in zip(raws[1], (qT_d, qR_d, kT_d, kR_d)):
                nc.sync.dma_start(out=tl_, in_=src.ap()[1])
            nc.sync.dma_start(out=VBF[1], in_=v_d.ap()[1])
            nc.sync.dma_start(out=zk1, in_=zk_d.ap()[1])
            nc.sync.dma_start(out=VGA[1][:, 0:1], in_=zv_d.ap()[1])
            zk_t = {0: zk0, 1: zk1}
            for h in range(HPC):
                nc.vector.tensor_copy(out=KGT[h][:, 0:1], in_=zk_t[h])

            for h in range(HPC):
                qraw, qrot, kraw, krot = raws[h]
                # ---------------- RoPE (DVE, bf16 2x mode)
                spans = ([(0, 512), (512, 1024)] if h == 0
                         else [(0, 1024)]) + [(c, c + 1024)
                                              for c in range(1024, T, 1024)]
                for c0, c1 in spans:
                    cs = slice(c0, c1)
                    csz = c1 - c0
                    for src, srcR, dst in ((qraw, qrot, QT[h]),
                                           (kraw, krot, KT[h])):
                        ta = ropetmp.tile([D, 1024], bf16, tag="ta", name="ta")
                        tb = ropetmp.tile([D, 1024], bf16, tag="tb", name="tb")
                        nc.vector.tensor_tensor(
                            out=ta[:, 0:csz], in0=src[:, cs],
                            in1=cosT[:, cs], op=MUL)
                        nc.vector.tensor_tensor(
                            out=tb[:, 0:csz], in0=srcR[:, cs],
                            in1=sinRT[:, cs], op=MUL)
                        nc.vector.tensor_tensor(
                            out=dst[:, cs], in0=ta[:, 0:csz],
                            in1=tb[:, 0:csz], op=ADD)

                nc.vector.memset(VGA[h][:, 1:KLEN], 0.0)

                kgf = persist.tile([D, KLEN], f32, tag=f"kgf{h}", name=f"kgf{h}")

                for p in range(PNUM):
                    bs = slice(p * L, (p + 1) * L)
                    gsl = slice(1 + 8 * p, 9 + 8 * p)
                    # ---------------- streamed global pooling for block p
                    # K pooling: sum over each 128-token group (gpsimd), then
                    # scale by 1/S into bf16 kgT.
                    nc.vector.tensor_reduce(
                        out=kgf[:, gsl],
                        in_=KT[h][:, bs].rearrange("p (g s) -> p g s", s=S),
                        axis=mybir.AxisListType.X,
                        op=ADD,
                    )
                    nc.vector.tensor_scalar_mul(
                        out=kgf[:, gsl], in0=kgf[:, gsl], scalar1=1.0 / S)
                    nc.vector.tensor_copy(
                        out=KGT[h][:, gsl], in_=kgf[:, gsl])
                    # V pooling via PE: one accumulation group, one col/grp
                    vgp = scores_p.tile([128, 1024], f32, tag="s", name="vgp")
                    for g in range(8):
                        nc.tensor.matmul(
                            out=vgp[:, g : g + 1],
                            lhsT=VBF[h][:, 8 * p + g, :],
                            rhs=poolcol,
                            start=(g == 0),
                            stop=(g == 7),
                        )
                    nc.vector.tensor_copy(out=VGA[h][:, gsl], in_=vgp[:, 0:8])
                    # rebuild Vg rows (token-major pooled V, bf16)
                    vgt = scores_p.tile([KLEN, 128], f32, tag="s", name="vgt")
                    nc.tensor.transpose(out=vgt, in_=VGA[h], identity=ident)
                    nc.vector.tensor_copy(out=VG[h], in_=vgt)

                    # ---------------- local block p
                    q0 = p * L
                    ms = list(range(1, 9)) if p == 0 else list(range(0, 9))
                    expt = {}
                    sums = acc_p.tile([128, 1024], f32, tag="sum", name="sums")
                    sum_started = [False, False]
                    sum_last_m = {
                        reg: max(
                            m for m in ms
                            if (0 if m == 0 else 128 * (m - 1)) < 512 * (reg + 1)
                        )
                        for reg in (0, 1)
                    }
                    for m in ms:
                        start_m = 0 if m == 0 else 128 * (m - 1)
                        kcol = q0 - 128 + 128 * m  # k-token start (abs)
                        st = scores_p.tile([128, 1024], f32, tag="s", name="st")
                        for r0 in range(start_m - start_m % 512, 1024, 512):
                            c_lo = max(start_m, r0)
                            c_hi = r0 + 512
                            is_diag_reg = m >= 1 and start_m >= r0
                            nc.tensor.matmul(
                                out=st[:, c_lo:c_hi],
                                lhsT=KT[h][:, kcol : kcol + 128],
                                rhs=QT[h][:, q0 + c_lo : q0 + c_hi],
                                start=True,
                                stop=not is_diag_reg,
                            )
                            if is_diag_reg:
                                nc.tensor.matmul(
                                    out=st[:, start_m : start_m + 128],
                                    lhsT=mB,
                                    rhs=mC,
                                    start=False,
                                    stop=True,
                                )
                        et = expp.tile([128, 1024], bf16, tag="e", name="et")
                        nc.scalar.activation(
                            out=et[:, start_m:1024],
                            in_=st[:, start_m:1024],
                            func=EXP,
                            scale=SCALE,
                        )
                        expt[m] = et
                        for reg in (0, 1):
                            c_lo = max(start_m, reg * 512)
                            c_hi = (reg + 1) * 512
                            if c_lo >= c_hi:
                                continue
                            nc.tensor.matmul(
                                out=sums[:, c_lo:c_hi],
                                lhsT=ones_bf,
                                rhs=et[:, c_lo:c_hi],
                                start=not sum_started[reg],
                                stop=(m == sum_last_m[reg]),
                            )
                            sum_started[reg] = True
                    # PV: O^T accumulation per 512-col region
                    ot = acc_p.tile([128, 1024], f32, tag="o", name="ot")
                    for reg in (0, 1):
                        valid_ms = [
                            m for m in ms
                            if (0 if m == 0 else 128 * (m - 1)) < 512 * (reg + 1)
                        ]
                        for i, m in enumerate(valid_ms):
                            start_m = 0 if m == 0 else 128 * (m - 1)
                            c_lo = max(start_m, reg * 512)
                            c_hi = (reg + 1) * 512
                            vidx = 8 * p - 1 + m
                            nc.tensor.matmul(
                                out=ot[:, c_lo:c_hi],
                                lhsT=VBF[h][:, vidx, :],
                                rhs=expt[m][:, c_lo:c_hi],
                                start=(i == 0),
                                stop=(m == valid_ms[-1]),
                            )
                    # normalize local branch (walrus: TensorTensor may read
                    # at most one PSUM operand -> recip to SBUF, then mul)
                    rl = combine.tile([128, 1024], f32, tag="rl", name="rl", bufs=1)
                    nc.vector.reciprocal(out=rl, in_=sums)
                    tl = combine.tile([128, 1024], bf16, tag="tl", name="tl")
                    nc.vector.tensor_tensor(out=tl, in0=ot, in1=rl, op=MUL)

                    # ---------------- global chunks 2p, 2p+1 (after local:
                    # psum tags are reused; scheduler overlaps with next block)
                    sg = scores_p.tile([128, 1024], f32, tag="s", name="sg")
                    eg = expp.tile([128, 1024], bf16, tag="e", name="eg")
                    for half, c in ((0, 2 * p), (1, 2 * p + 1)):
                        rows = min(KLEN, 4 * c + 4)
                        qs = slice(c * CHUNK, (c + 1) * CHUNK)
                        col = slice(half * CHUNK, (half + 1) * CHUNK)
                        nc.tensor.matmul(
                            out=sg[0:rows, col],
                            lhsT=KGT[h][:, 0:rows],
                            rhs=QT[h][:, qs],
                            start=True,
                            stop=False,
                        )
                        nc.tensor.matmul(
                            out=sg[0:rows, col],
                            lhsT=gB[:, KLEN * c : KLEN * c + rows],
                            rhs=gC,
                            start=False,
                            stop=True,
                        )
                        nc.scalar.activation(
                            out=eg[0:rows, col],
                            in_=sg[0:rows, col],
                            func=EXP,
                            scale=SCALE,
                        )
                    gs = acc_p.tile([128, 1024], f32, tag="sum", name="gs")
                    go = acc_p.tile([128, 1024], f32, tag="o", name="go")
                    for half, c in ((0, 2 * p), (1, 2 * p + 1)):
                        rows = min(KLEN, 4 * c + 4)
                        col = slice(half * CHUNK, (half + 1) * CHUNK)
                        nc.tensor.matmul(
                            out=gs[:, col],
                            lhsT=ones_bf[0:rows, :],
                            rhs=eg[0:rows, col],
                            start=True,
                            stop=True,
                        )
                        nc.tensor.matmul(
                            out=go[:, col],
                            lhsT=VG[h][0:rows, :],
                            rhs=eg[0:rows, col],
                            start=True,
                            stop=True,
                        )
                    # normalize global branch + final add + output DMA.
                    # Last block of the last head: process in 512-halves so
                    # the tail chain pipelines into the output DMA.
                    rg = combine.tile([128, 1024], f32, tag="rg", name="rg", bufs=1)
                    ogn = combine.tile([128, 1024], bf16, tag="ogn", name="ogn")
                    halves = ((slice(0, 512), slice(p * L, p * L + 512)),
                              (slice(512, 1024),
                               slice(p * L + 512, (p + 1) * L)))
                    if h == HPC - 1 and p == PNUM - 1:
                        for hv, obs in halves:
                            nc.vector.reciprocal(out=rg[:, hv], in_=gs[:, hv])
                            nc.vector.tensor_tensor(
                                out=ogn[:, hv], in0=go[:, hv], in1=rg[:, hv],
                                op=MUL)
                            nc.vector.tensor_tensor(
                                out=OUTSB[h][:, obs], in0=tl[:, hv],
                                in1=ogn[:, hv], op=ADD)
                            nc.sync.dma_start(
                                out=out_d.ap()[h][:, obs],
                                in_=OUTSB[h][:, obs])
                    else:
                        nc.vector.reciprocal(out=rg, in_=gs)
                        nc.vector.tensor_tensor(out=ogn, in0=go, in1=rg, op=MUL)
                        nc.vector.tensor_tensor(
                            out=OUTSB[h][:, bs], in0=tl, in1=ogn, op=ADD)
                        nc.sync.dma_start(
                            out=out_d.ap()[h][:, bs], in_=OUTSB[h][:, bs])
    return nc


def _get_program():
    if "nc" not in _CACHE:
        _CACHE["nc"] = _build_program()
        _CACHE["consts"] = _host_constants()
    return _CACHE["nc"], _CACHE["consts"]


def _prepare_in_maps(q, k, v, zero_k, zero_v):
    _, consts = _get_program()
    q4 = np.asarray(q, dtype=np.float32).reshape(T, H, D)
    k4 = np.asarray(k, dtype=np.float32).reshape(T, H, D)
    v4 = np.asarray(v, dtype=np.float32).reshape(T, H, D)
    zk = np.asarray(zero_k, dtype=np.float32).reshape(H, D)
    zv = np.asarray(zero_v, dtype=np.float32).reshape(H, D)

    in_maps = []
    for core in range(NCORES):
        hs = slice(HPC * core, HPC * (core + 1))
        qT = np.ascontiguousarray(q4[:, hs].transpose(1, 2, 0)).astype(_BF)
        kT = np.ascontiguousarray(k4[:, hs].transpose(1, 2, 0)).astype(_BF)
        qR = np.ascontiguousarray(np.concatenate(
            [qT[:, 64:128], qT[:, 0:64]], axis=1))
        kR = np.ascontiguousarray(np.concatenate(
            [kT[:, 64:128], kT[:, 0:64]], axis=1))
        # v partition-major: vP[h, p, n, d] = v[n*128 + p, h, d]
        vP = np.ascontiguousarray(
            v4[:, hs].reshape(NT, 128, HPC, D).transpose(2, 1, 0, 3)
        ).astype(_BF)
        in_maps.append(
            {
                "qT": qT,
                "kT": kT,
                "qR": qR,
                "kR": kR,
                "vP": vP,
                "zk": np.ascontiguousarray(zk[hs])[:, :, None],
                "zv": np.ascontiguousarray(zv[hs])[:, :, None],
                **consts,
            }
        )
    return in_maps


def _assemble(results):
    # outT per core: [HPC, D, T] bf16 -> out[t, 0, (HPC*core+h)*D + d]
    arr = np.stack([np.asarray(results[i]["outT"]) for i in range(NCORES)])
    out = arr.astype(np.float32).transpose(3, 0, 1, 2).reshape(T, 1, H * D)
    return np.ascontiguousarray(out)


# ---------------------------------------------------------------- entry point
def kernel(q, k, v, zero_k, zero_v):
    nc, _ = _get_program()
    from concourse.bass_utils import run_bass_kernel_spmd

    in_maps = _prepare_in_maps(q, k, v, zero_k, zero_v)
    res = run_bass_kernel_spmd(nc, in_maps, core_ids=list(range(NCORES)))
    return _assemble([res.results[i] for i in range(NCORES)])


# revision 4
# speedup vs baseline: 1.0374x; 1.0374x over previous
"""Trainium2 Bass kernel for CoreAttentionExpand (sparse local+global attention).

Sharding: tensor-parallel over heads. 16 heads / 8 cores = 2 heads per core.
Each core computes RoPE + local-block attention + pooled-global attention for
its 2 heads end-to-end (no collectives); host reassembles the full output.

Layout notes (vs. the original f32 kernel, ~181us -> ~141us in TimelineSim):
- All device data is bf16 (host pre-casts); scores/PV accumulate in f32 PSUM.
  Error budget: rel err ~5e-3 vs the 2e-2 tolerance.
- RoPE = 3 bf16 tensor_tensor ops on DVE per chunk (2x DVE mode); the
  rotate-half operand is a host-prepared HBM copy (no on-chip shuffle).
- Softmax normalization: reciprocal (PSUM->SBUF) + multiply; walrus allows at
  most one PSUM operand per TensorTensor, and gpsimd may not touch PSUM.
- V is loaded partition-major (full-speed DMA descriptors); K/V global pooling
  is streamed per block (causal); per-block output DMAs shrink the tail.
- Output is bf16 [head, D, T]; host converts to f32 and reassembles.
"""

import sys
import math

_REPO = "/opt/trn_rl_repo"
if _REPO not in sys.path:
    sys.path.insert(0, _REPO)

import numpy as np
import ml_dtypes

# ---------------------------------------------------------------- constants
H = 16          # heads
D = 128         # head dim
T = 4096        # tokens
L = 1024        # local block size
S = 128         # global pool stride
E = 128         # local history size
PNUM = T // L   # 4 local blocks
KLEN = T // S + 1  # 33 global keys (incl. zero token)
NT = T // 128   # 32 token-tiles per head
NCORES = 8
HPC = H // NCORES  # 2 heads per core
NEGBIG = -10000.0
SCALE = 1.0 / math.sqrt(D)
CHUNK = 512     # global q-chunk width

_CACHE = {}
_BF = ml_dtypes.bfloat16


def _apply_framework_patches(bassmod, mybir, tilemod):
    """This walrus build rejects >1 sem wait per instruction; split excess
    waits onto preceding same-engine NoOps (pure scheduling transform)."""
    if getattr(tilemod.TileContext, "_wait_split_patched", False):
        return
    TileContext = tilemod.TileContext
    ScopedClock = tilemod.ScopedClock

    orig_add = TileContext._add_instruction
    ctr = [0]

    def split_add(self, inst):
        si = inst.sync_info
        if si is not None and si.on_wait and len(si.on_wait) > 1:
            ow = list(si.on_wait)
            for w in ow[:-1]:
                ctr[0] += 1
                nop = mybir.InstNoOp(name=f"I-wsplit{ctr[0]}", engine=inst.engine)
                nop.sync_info = mybir.SyncInfo(on_wait=[w], on_update=[])
                orig_add(self, nop)
            si.on_wait = [ow[-1]]
        orig_add(self, inst)

    def drain_and_barrier(self, tick_clock, wait_clock):
        nc = self.nc
        probe = nc.sync.nop(nofuse=True, hint="waitprobe")
        wait_clock.add_sem_waits(
            probe.ins, ScopedClock({None: tick_clock.global_clock})
        )
        si = probe.ins.sync_info
        ow = list(si.on_wait) if si and si.on_wait else []
        if len(ow) > 1:
            si.on_wait = ow[:1]
            for w in ow[1:]:
                n2 = nc.sync.nop(nofuse=True, hint="waitsplit")
                n2.ins.sync_info = mybir.SyncInfo(on_wait=[w], on_update=[])
        nc.sync.drain()
        nc.all_engine_barrier()
        popped = nc._tile_sem_poison_stack.pop()
        assert popped is self._sem_poison
        nc.clear_and_free_semaphores(list(self.sems.allocated().values()))
        nc.all_engine_barrier()

    TileContext._add_instruction = split_add
    TileContext._drain_and_barrier = drain_and_barrier
    TileContext._wait_split_patched = True


# ---------------------------------------------------------------- constants (host)
def _host_constants():
    t = np.arange(T, dtype=np.float32)
    inv = (1.0 / (10000.0 ** (np.arange(0, D, 2, dtype=np.float32) / D))).astype(
        np.float32
    )  # [64]
    emb = t[:, None] * inv[None, :]          # [T, 64]
    cos64 = np.cos(emb).astype(np.float32)
    sin64 = np.sin(emb).astype(np.float32)
    # [D, T] head-dim-major tables (sign of rotate-half folded into sinRT)
    cosT = np.ascontiguousarray(np.concatenate([cos64, cos64], axis=1).T).astype(_BF)
    sinRT = np.ascontiguousarray(np.concatenate([-sin64, sin64], axis=1).T).astype(_BF)

    idx = np.arange(128)
    # causal ramp: (mB^T mC)[k, q] = NEGBIG * max(k - q, 0)
    mB = (idx[:, None] <= idx[None, :]).astype(_BF)              # [m, k]: m <= k
    mC = (NEGBIG * (idx[:, None] > idx[None, :])).astype(_BF)    # [m, q]: m > q
    ones_bf = np.ones((128, 128), dtype=_BF)
    # global stairstep: for chunk c, row j, masked cols qq < 128*(j - 4c)
    gB = np.zeros((3, 8 * KLEN), dtype=np.float32)
    for c in range(8):
        for mm in range(3):
            for j in range(KLEN):
                gB[mm, KLEN * c + j] = 1.0 if j >= 4 * c + mm + 1 else 0.0
    gB = gB.astype(_BF)
    qq = np.arange(CHUNK)
    gC = np.stack(
        [NEGBIG * ((qq >= 128 * mm) & (qq < 128 * (mm + 1))) for mm in range(3)]
    ).astype(_BF)                                                # [3, 512]
    ident = np.eye(128, dtype=np.float32)
    poolcol = np.full((128, 1), 1.0 / S, dtype=_BF)              # pooling matmul rhs
    return {
        "cosT": cosT,
        "sinRT": sinRT,
        "mB": mB,
        "mC": mC,
        "ones_bf": ones_bf,
        "gB": gB,
        "gC": gC,
        "ident": ident,
        "poolcol": poolcol,
    }


# ---------------------------------------------------------------- device program
def _build_program():
    import concourse.bass as bass
    import concourse.mybir as mybir
    import concourse.tile as tile

    _apply_framework_patches(bass, mybir, tile)

    f32 = mybir.dt.float32
    bf16 = mybir.dt.bfloat16
    EXP = mybir.ActivationFunctionType.Exp
    MUL = mybir.AluOpType.mult
    ADD = mybir.AluOpType.add
    DIV = mybir.AluOpType.divide

    nc = bass.Bass()
    qT_d = nc.dram_tensor("qT", [HPC, D, T], bf16, kind="ExternalInput")
    qR_d = nc.dram_tensor("qR", [HPC, D, T], bf16, kind="ExternalInput")
    kT_d = nc.dram_tensor("kT", [HPC, D, T], bf16, kind="ExternalInput")
    kR_d = nc.dram_tensor("kR", [HPC, D, T], bf16, kind="ExternalInput")
    v_d = nc.dram_tensor("vP", [HPC, 128, NT, D], bf16, kind="ExternalInput")
    zk_d = nc.dram_tensor("zk", [HPC, D, 1], f32, kind="ExternalInput")
    zv_d = nc.dram_tensor("zv", [HPC, D, 1], f32, kind="ExternalInput")
    cosT_d = nc.dram_tensor("cosT", [D, T], bf16, kind="ExternalInput")
    sinRT_d = nc.dram_tensor("sinRT", [D, T], bf16, kind="ExternalInput")
    mB_d = nc.dram_tensor("mB", [128, 128], bf16, kind="ExternalInput")
    mC_d = nc.dram_tensor("mC", [128, 128], bf16, kind="ExternalInput")
    ones_d = nc.dram_tensor("ones_bf", [128, 128], bf16, kind="ExternalInput")
    gB_d = nc.dram_tensor("gB", [3, 8 * KLEN], bf16, kind="ExternalInput")
    gC_d = nc.dram_tensor("gC", [3, CHUNK], bf16, kind="ExternalInput")
    poolcol_d = nc.dram_tensor("poolcol", [128, 1], bf16, kind="ExternalInput")
    ident_d = nc.dram_tensor("ident", [128, 128], f32, kind="ExternalInput")
    out_d = nc.dram_tensor("outT", [HPC, D, T], bf16, kind="ExternalOutput")

    with tile.TileContext(nc) as tc:
        with (
            tc.tile_pool(name="persist", bufs=1) as persist,
            tc.tile_pool(name="ropebuf", bufs=2) as ropebuf,
            tc.tile_pool(name="ropetmp", bufs=2) as ropetmp,
            tc.tile_pool(name="expp", bufs=12) as expp,
            tc.tile_pool(name="small", bufs=2) as small,
            tc.tile_pool(name="combine", bufs=2) as combine,
            tc.tile_pool(name="scores", bufs=2, space="PSUM") as scores_p,
            tc.tile_pool(name="acc", bufs=1, space="PSUM") as acc_p,
        ):
            # ---- tiles
            mB = persist.tile([128, 128], bf16, tag="mB")
            mC = persist.tile([128, 128], bf16, tag="mC")
            ones_bf = persist.tile([128, 128], bf16, tag="ones")
            gB = persist.tile([3, 8 * KLEN], bf16, tag="gB")
            gC = persist.tile([3, CHUNK], bf16, tag="gC")
            poolcol = persist.tile([128, 1], bf16, tag="poolcol")
            ident = persist.tile([128, 128], f32, tag="ident")
            zk0 = persist.tile([D, 1], f32, tag="zk0")
            zk1 = persist.tile([D, 1], f32, tag="zk1")

            QT, KT, VBF, KGT, VGA, VG, OUTSB = {}, {}, {}, {}, {}, {}, {}
            for h in range(HPC):
                QT[h] = persist.tile([D, T], bf16, tag=f"QT{h}", name=f"QT{h}")
                KT[h] = persist.tile([D, T], bf16, tag=f"KT{h}", name=f"KT{h}")
                VBF[h] = persist.tile([128, NT, D], bf16, tag=f"vbf{h}",
                                      name=f"VBF{h}")
                KGT[h] = persist.tile([D, KLEN], bf16, tag=f"kgT{h}",
                                      name=f"KGT{h}")
                VGA[h] = persist.tile([D, KLEN], f32, tag=f"vgA{h}",
                                      name=f"VGA{h}")
                VG[h] = persist.tile([KLEN, 128], bf16, tag=f"Vg{h}",
                                     name=f"VG{h}")
                OUTSB[h] = persist.tile([D, T], bf16, tag=f"out{h}",
                                        name=f"OUTSB{h}")
            # per-chunk raw/table tiles: ONE full DMA per tile (proven
            # pattern; multi-piece writes into one tile broke on HW)
            cosc, sinc, rawc = {}, {}, {}
            for c in range(4):
                cosc[c] = persist.tile([D, 1024], bf16, tag=f"cosc{c}",
                                       name=f"cosc{c}")
                sinc[c] = persist.tile([D, 1024], bf16, tag=f"sinc{c}",
                                       name=f"sinc{c}")
            for h in range(HPC):
                for c in range(4):
                    rawc[(h, c)] = tuple(
                        ropebuf.tile([D, 1024], bf16, tag=f"{nm}{c}",
                                     name=f"{nm}{h}_{c}")
                        for nm in ("qraw", "qrot", "kraw", "krot"))

            # ---- DMA issue order: h0 block-0 critical path first
            def chunk_dmas(h, c):
                cs = slice(c * 1024, (c + 1) * 1024)
                if h == 0:
                    nc.sync.dma_start(out=cosc[c], in_=cosT_d.ap()[:, cs])
                    nc.sync.dma_start(out=sinc[c], in_=sinRT_d.ap()[:, cs])
                for tl_, src in zip(rawc[(h, c)], (qT_d, qR_d, kT_d, kR_d)):
                    nc.sync.dma_start(out=tl_, in_=src.ap()[h][:, cs])

            chunk_dmas(0, 0)
            nc.sync.dma_start(out=mB, in_=mB_d.ap())
            nc.sync.dma_start(out=mC, in_=mC_d.ap())
            nc.sync.dma_start(out=ones_bf, in_=ones_d.ap())
            nc.sync.dma_start(out=VBF[0], in_=v_d.ap()[0])
            chunk_dmas(0, 1)
            nc.sync.dma_start(out=gB, in_=gB_d.ap())
            nc.sync.dma_start(out=gC, in_=gC_d.ap())
            nc.sync.dma_start(out=poolcol, in_=poolcol_d.ap())
            nc.sync.dma_start(out=ident, in_=ident_d.ap())
            nc.sync.dma_start(out=zk0, in_=zk_d.ap()[0])
            nc.sync.dma_start(out=VGA[0][:, 0:1], in_=zv_d.ap()[0])
            chunk_dmas(0, 2)
            chunk_dmas(0, 3)
            for c in range(4):
                chunk_dmas(1, c)
            nc.sync.dma_start(out=VBF[1], in_=v_d.ap()[1])
            nc.sync.dma_start(out=zk1, in_=zk_d.ap()[1])
            nc.sync.dma_start(out=VGA[1][:, 0:1], in_=zv_d.ap()[1])
            zk_t = {0: zk0, 1: zk1}
            for h in range(HPC):
                nc.vector.tensor_copy(out=KGT[h][:, 0:1], in_=zk_t[h])

            for h in range(HPC):
                nc.vector.memset(VGA[h][:, 1:KLEN], 0.0)

                kgf = persist.tile([D, KLEN], f32, tag=f"kgf{h}", name=f"kgf{h}")

                for p in range(PNUM):
                    # ---------------- RoPE for this block (DVE, bf16 2x)
                    cs = slice(p * 1024, (p + 1) * 1024)
                    qrawc, qrotc, krawc, krotc = rawc[(h, p)]
                    for srcT, srcR, dst in ((krawc, krotc, KT[h]),
                                            (qrawc, qrotc, QT[h])):
                        ta = ropetmp.tile([D, 1024], bf16, tag="ta", name="ta")
                        tb = ropetmp.tile([D, 1024], bf16, tag="tb", name="tb")
                        nc.vector.tensor_tensor(
                            out=ta, in0=srcT, in1=cosc[p], op=MUL)
                        nc.vector.tensor_tensor(
                            out=tb, in0=srcR, in1=sinc[p], op=MUL)
                        nc.vector.tensor_tensor(
                            out=dst[:, cs], in0=ta, in1=tb, op=ADD)
                    bs = slice(p * L, (p + 1) * L)
                    gsl = slice(1 + 8 * p, 9 + 8 * p)

                    # ---------------- local block p
                    q0 = p * L
                    ms = list(range(1, 9)) if p == 0 else list(range(0, 9))
                    expt = {}
                    sums = acc_p.tile([128, 1024], f32, tag="sum", name="sums")
                    sum_started = [False, False]
                    sum_last_m = {
                        reg: max(
                            m for m in ms
                            if (0 if m == 0 else 128 * (m - 1)) < 512 * (reg + 1)
                        )
                        for reg in (0, 1)
                    }
                    for m in ms:
                        start_m = 0 if m == 0 else 128 * (m - 1)
                        kcol = q0 - 128 + 128 * m  # k-token start (abs)
                        st = scores_p.tile([128, 1024], f32, tag="s", name="st")
                        for r0 in range(start_m - start_m % 512, 1024, 512):
                            c_lo = max(start_m, r0)
                            c_hi = r0 + 512
                            is_diag_reg = m >= 1 and start_m >= r0
                            nc.tensor.matmul(
                                out=st[:, c_lo:c_hi],
                                lhsT=KT[h][:, kcol : kcol + 128],
                                rhs=QT[h][:, q0 + c_lo : q0 + c_hi],
                                start=True,
                                stop=not is_diag_reg,
                            )
                            if is_diag_reg:
                                nc.tensor.matmul(
                                    out=st[:, start_m : start_m + 128],
                                    lhsT=mB,
                                    rhs=mC,
                                    start=False,
                                    stop=True,
                                )
                        et = expp.tile([128, 1024], bf16, tag="e", name="et")
                        nc.scalar.activation(
                            out=et[:, start_m:1024],
                            in_=st[:, start_m:1024],
                            func=EXP,
                            scale=SCALE,
                        )
                        expt[m] = et
                        for reg in (0, 1):
                            c_lo = max(start_m, reg * 512)
                            c_hi = (reg + 1) * 512
                            if c_lo >= c_hi:
                                continue
                            nc.tensor.matmul(
                                out=sums[:, c_lo:c_hi],
                                lhsT=ones_bf,
                                rhs=et[:, c_lo:c_hi],
                                start=not sum_started[reg],
                                stop=(m == sum_last_m[reg]),
                            )
                            sum_started[reg] = True
                    # PV: O^T accumulation per 512-col region
                    ot = acc_p.tile([128, 1024], f32, tag="o", name="ot")
                    for reg in (0, 1):
                        valid_ms = [
                            m for m in ms
                            if (0 if m == 0 else 128 * (m - 1)) < 512 * (reg + 1)
                        ]
                        for i, m in enumerate(valid_ms):
                            start_m = 0 if m == 0 else 128 * (m - 1)
                            c_lo = max(start_m, reg * 512)
                            c_hi = (reg + 1) * 512
                            vidx = 8 * p - 1 + m
                            nc.tensor.matmul(
                                out=ot[:, c_lo:c_hi],
                                lhsT=VBF[h][:, vidx, :],
                                rhs=expt[m][:, c_lo:c_hi],
                                start=(i == 0),
                                stop=(m == valid_ms[-1]),
                            )
                    # normalize local branch (walrus: TensorTensor may read
                    # at most one PSUM operand -> recip to SBUF, then mul)
                    rl = combine.tile([128, 1024], f32, tag="rl", name="rl", bufs=1)
                    nc.vector.reciprocal(out=rl, in_=sums)
                    tl = combine.tile([128, 1024], bf16, tag="tl", name="tl")
                    nc.vector.tensor_tensor(out=tl, in0=ot, in1=rl, op=MUL)

                    # ---------------- streamed global pooling for block p
                    # (emitted after the local m-loop: only the global branch
                    # consumes it, and early emission stalls in-order queues)
                    nc.vector.tensor_reduce(
                        out=kgf[:, gsl],
                        in_=KT[h][:, bs].rearrange("p (g s) -> p g s", s=S),
                        axis=mybir.AxisListType.X,
                        op=ADD,
                    )
                    nc.vector.tensor_scalar_mul(
                        out=kgf[:, gsl], in0=kgf[:, gsl], scalar1=1.0 / S)
                    nc.vector.tensor_copy(
                        out=KGT[h][:, gsl], in_=kgf[:, gsl])
                    vgp = scores_p.tile([128, 1024], f32, tag="s", name="vgp")
                    for g in range(8):
                        nc.tensor.matmul(
                            out=vgp[:, g : g + 1],
                            lhsT=VBF[h][:, 8 * p + g, :],
                            rhs=poolcol,
                            start=(g == 0),
                            stop=(g == 7),
                        )
                    nc.vector.tensor_copy(out=VGA[h][:, gsl], in_=vgp[:, 0:8])
                    vgt = scores_p.tile([KLEN, 128], f32, tag="s", name="vgt")
                    nc.tensor.transpose(out=vgt, in_=VGA[h], identity=ident)
                    nc.vector.tensor_copy(out=VG[h], in_=vgt)

                    # ---------------- global chunks 2p, 2p+1 (after local:
                    # psum tags are reused; scheduler overlaps with next block)
                    sg = scores_p.tile([128, 1024], f32, tag="s", name="sg")
                    eg = expp.tile([128, 1024], bf16, tag="e", name="eg")
                    for half, c in ((0, 2 * p), (1, 2 * p + 1)):
                        rows = min(KLEN, 4 * c + 4)
                        qs = slice(c * CHUNK, (c + 1) * CHUNK)
                        col = slice(half * CHUNK, (half + 1) * CHUNK)
                        nc.tensor.matmul(
                            out=sg[0:rows, col],
                            lhsT=KGT[h][:, 0:rows],
                            rhs=QT[h][:, qs],
                            start=True,
                            stop=False,
                        )
                        nc.tensor.matmul(
                            out=sg[0:rows, col],
                            lhsT=gB[:, KLEN * c : KLEN * c + rows],
                            rhs=gC,
                            start=False,
                            stop=True,
                        )
                        nc.scalar.activation(
                            out=eg[0:rows, col],
                            in_=sg[0:rows, col],
                            func=EXP,
                            scale=SCALE,
                        )
                    gs = acc_p.tile([128, 1024], f32, tag="sum", name="gs")
                    go = acc_p.tile([128, 1024], f32, tag="o", name="go")
                    for half, c in ((0, 2 * p), (1, 2 * p + 1)):
                        rows = min(KLEN, 4 * c + 4)
                        col = slice(half * CHUNK, (half + 1) * CHUNK)
                        nc.tensor.matmul(
                            out=gs[:, col],
                            lhsT=ones_bf[0:rows, :],
                            rhs=eg[0:rows, col],
                            start=True,
                            stop=True,
                        )
                        nc.tensor.matmul(
                            out=go[:, col],
                            lhsT=VG[h][0:rows, :],
                            rhs=eg[0:rows, col],
                            start=True,
                            stop=True,
                        )
                    # normalize global branch + final add + output DMA.
                    # Last block of the last head: process in 512-halves so
                    # the tail chain pipelines into the output DMA.
                    rg = combine.tile([128, 1024], f32, tag="rg", name="rg", bufs=1)
                    ogn = combine.tile([128, 1024], bf16, tag="ogn", name="ogn")
                    halves = ((slice(0, 512), slice(p * L, p * L + 512)),
                              (slice(512, 1024),
                               slice(p * L + 512, (p + 1) * L)))
                    if h == HPC - 1 and p == PNUM - 1:
                        for hv, obs in halves:
                            nc.vector.reciprocal(out=rg[:, hv], in_=gs[:, hv])
                            nc.vector.tensor_tensor(
                                out=ogn[:, hv], in0=go[:, hv], in1=rg[:, hv],
                                op=MUL)
                            nc.vector.tensor_tensor(
                                out=OUTSB[h][:, obs], in0=tl[:, hv],
                                in1=ogn[:, hv], op=ADD)
                            nc.sync.dma_start(
                                out=out_d.ap()[h][:, obs],
                                in_=OUTSB[h][:, obs])
                    else:
                        nc.vector.reciprocal(out=rg, in_=gs)
                        nc.vector.tensor_tensor(out=ogn, in0=go, in1=rg, op=MUL)
                        nc.vector.tensor_tensor(
                            out=OUTSB[h][:, bs], in0=tl, in1=ogn, op=ADD)
                        nc.sync.dma_start(
                            out=out_d.ap()[h][:, bs], in_=OUTSB[h][:, bs])
    return nc


def _get_program():
    if "nc" not in _CACHE:
        _CACHE["nc"] = _build_program()
        _CACHE["consts"] = _host_constants()
    return _CACHE["nc"], _CACHE["consts"]


def _prepare_in_maps(q, k, v, zero_k, zero_v):
    _, consts = _get_program()
    q4 = np.asarray(q, dtype=np.float32).reshape(T, H, D)
    k4 = np.asarray(k, dtype=np.float32).reshape(T, H, D)
    v4 = np.asarray(v, dtype=np.float32).reshape(T, H, D)
    zk = np.asarray(zero_k, dtype=np.float32).reshape(H, D)
    zv = np.asarray(zero_v, dtype=np.float32).reshape(H, D)

    in_maps = []
    for core in range(NCORES):
        hs = slice(HPC * core, HPC * (core + 1))
        qT = np.ascontiguousarray(q4[:, hs].transpose(1, 2, 0)).astype(_BF)
        kT = np.ascontiguousarray(k4[:, hs].transpose(1, 2, 0)).astype(_BF)
        qR = np.ascontiguousarray(np.concatenate(
            [qT[:, 64:128], qT[:, 0:64]], axis=1))
        kR = np.ascontiguousarray(np.concatenate(
            [kT[:, 64:128], kT[:, 0:64]], axis=1))
        # v partition-major: vP[h, p, n, d] = v[n*128 + p, h, d]
        vP = np.ascontiguousarray(
            v4[:, hs].reshape(NT, 128, HPC, D).transpose(2, 1, 0, 3)
        ).astype(_BF)
        in_maps.append(
            {
                "qT": qT,
                "kT": kT,
                "qR": qR,
                "kR": kR,
                "vP": vP,
                "zk": np.ascontiguousarray(zk[hs])[:, :, None],
                "zv": np.ascontiguousarray(zv[hs])[:, :, None],
                **consts,
            }
        )
    return in_maps


def _assemble(results):
    # outT per core: [HPC, D, T] bf16 -> out[t, 0, (HPC*core+h)*D + d]
    arr = np.stack([np.asarray(results[i]["outT"]) for i in range(NCORES)])
    out = arr.astype(np.float32).transpose(3, 0, 1, 2).reshape(T, 1, H * D)
    return np.ascontiguousarray(out)


# ---------------------------------------------------------------- entry point
def kernel(q, k, v, zero_k, zero_v):
    nc, _ = _get_program()
    from concourse.bass_utils import run_bass_kernel_spmd

    in_maps = _prepare_in_maps(q, k, v, zero_k, zero_v)
    res = run_bass_kernel_spmd(nc, in_maps, core_ids=list(range(NCORES)))
    return _assemble([res.results[i] for i in range(NCORES)])
